# revision 14
# baseline (speedup 1.0000x reference)
"""Single-head attention (B=4, S=4096, E=1024, D=64) on 8 TRN2 NeuronCores.

Sharding: data-parallel over (batch, query-half): core c handles batch
b = c // 2 and query rows [h*2048, (h+1)*2048) with h = c % 2. Each core
computes Q for its own 2048 rows and K/V for the full 4096 rows of its batch.

Design (vs the v1 kernel this evolved from):
  * Row-tiled scores: the scores matmul contracts over D=64 (half the PE
    array), so two k-tiles run CONCURRENTLY as PE row-tiles (0,0)/(64,0),
    with lhsT/rhs at base partitions 0 and 64 (tile_position auto-derives).
    K is produced at BOTH bases for free by packing the projections as
    A=[Wk|Wq]own, C=[Wv|Wk]own (pass C's top half was idle in v1) and
    B=[Wk|Wv]oth, plus 8 small base-shifts for odd other-half tiles (kos).
    The (64,0) tiles consume Q at base 64 straight from the pass-A layout;
    the (0,0) tiles consume the base-0 shifted copy (qts).
  * Paired exp: each k-tile pair's scores land in one [128, 2048] PSUM
    tile (4 banks) -> ONE ScalarE activation per 2 k-tiles, amortizing the
    ~485ns per-activation overhead. PSUM: pair(4) + single(2) + att(2)
    banks; groups of 3 k-tiles = one pair + one single, double-buffered
    through the two scores slots. ScalarE: ~85.6us -> ~67us.
  * Per group the PE issues: pair scores, [side work], single scores, and
    the PREVIOUS group's AV matmuls. Side work (projection halves and
    PE-transpose bursts for V) is issued BEFORE the single so its PSUM
    allocation waits only on the previous group's single-exp; issuing it
    after would head-of-line-block the PE FIFO on the current exp.
  * Own-half k-tiles first within each pass so compute starts once the own
    half of x lands (~13us); the other half streams in under the loop.
  * HAM p-state care: warm-up transposes cover the DMA spin-up, heater
    transposes bridge piece-arrival gaps inside the prologue projections
    and the exp-paced pass-1 groups (sub-us PE idle slivers otherwise
    oscillate the clock down to 4/8).
  * attn rides with a ones-row in the AV stationary ([V|1], M=65): row 64
    accumulates softmax denominators; the host divides and transposes.
"""

import numpy as np

B, S, E, D = 4, 4096, 1024, 64
HALF = S // 2
N_CORES = 8
SCALE = 1.0 / np.sqrt(D)

NE = E // 128  # 8 e-tiles (contraction)
NT = S // 128  # 32 k-tiles per batch
N_WARM = 40

_CACHE = {}


def _build():
    if "nc" in _CACHE:
        return _CACHE["nc"]

    from contextlib import ExitStack

    import concourse.bacc as bacc
    import concourse.tile as tile
    from concourse import mybir
    from concourse.masks import make_identity

    FP32 = mybir.dt.float32
    BF16 = mybir.dt.bfloat16
    Exp = mybir.ActivationFunctionType.Exp

    nc = bacc.Bacc(
        "TRN2", target_bir_lowering=False, debug=False, num_devices=N_CORES
    )

    xt_q_d = nc.dram_tensor("xt_q", [E, HALF], BF16, kind="ExternalInput").ap()
    xt_o_d = nc.dram_tensor("xt_o", [E, HALF], BF16, kind="ExternalInput").ap()
    wt_d = nc.dram_tensor("wt", [E, 384], BF16, kind="ExternalInput").ap()
    out_d = nc.dram_tensor("out", [D + 1, HALF], FP32, kind="ExternalOutput").ap()

    with tile.TileContext(nc) as tc, ExitStack() as ctx:
        const = ctx.enter_context(tc.tile_pool(name="const", bufs=1))
        big = ctx.enter_context(tc.tile_pool(name="big", bufs=1))
        ppP = ctx.enter_context(tc.tile_pool(name="ppP", bufs=2))
        ppS = ctx.enter_context(tc.tile_pool(name="ppS", bufs=2))
        psP = ctx.enter_context(tc.tile_pool(name="psP", bufs=1, space="PSUM"))
        psS = ctx.enter_context(tc.tile_pool(name="psS", bufs=1, space="PSUM"))
        psA = ctx.enter_context(tc.tile_pool(name="psA", bufs=1, space="PSUM"))

        identB = const.tile([128, 128], BF16)
        make_identity(nc, identB)

        xt = big.tile([128, NE, S], BF16)  # x^T; cols [0, HALF) own
        wts = big.tile([128, NE, 384], BF16)  # [Wk|Wq | Wk|Wv | Wv|Wk]
        qk = big.tile([128, HALF], BF16)  # K_own@0-63 | Q_own@64-127
        ck = big.tile([128, HALF], BF16)  # V_own@0-63 | K_own@64-127
        kv = big.tile([128, HALF], BF16)  # K_oth@0-63 | V_oth@64-127
        qts = big.tile([64, HALF], BF16)  # Q_own shifted to base 0
        kos = big.tile([128, 8, 128], BF16)  # rows 64:128 = K_oth odd tiles
        vn = big.tile([128, NT, D + 1], BF16)  # V natural + ones col
        att_sb = big.tile([65, HALF], FP32)

        # --- PE warm-up over the DMA spin-up window ---
        warm = psS.tile([128, 1024], BF16, tag="s")
        for _ in range(N_WARM):
            nc.tensor.transpose(
                out=warm[0:128, 0:128], in_=identB[:, :], identity=identB[:, :]
            )

        # --- input DMAs: priority order per engine queue ---
        wtr = wt_d.rearrange("(t p) d -> p t d", p=128)

        def in_piece(eng, et, c0, c1, oth):
            src, off = (xt_o_d, HALF) if oth else (xt_q_d, 0)
            eng.dma_start(
                out=xt[:, et, off + c0 : off + c1],
                in_=src[et * 128 : (et + 1) * 128, c0:c1],
            )

        # ones columns of vn (vector queue is free until the first proj copy)
        for t in range(NT):
            nc.vector.memset(vn[:, t, D : D + 1], 1.0)

        nc.sync.dma_start(out=wts[:, :, 0:128], in_=wtr[:, :, 0:128])  # pack A
        first_engs = [nc.gpsimd, nc.sync, nc.scalar]
        # own cols 0:1024 in 512-col pieces (gates prologue + pass-0 q)
        for blk in range(2):
            for et in range(NE):
                in_piece(first_engs[et % 3], et, blk * 512, blk * 512 + 512, False)
        nc.gpsimd.dma_start(out=wts[:, :, 256:384], in_=wtr[:, :, 256:384])  # pack C
        late_engs = [nc.gpsimd, nc.sync]
        # own cols 1024:2048 (gates A2/A3/C2/C3 + pass-1 q)
        for et in range(NE):
            in_piece(late_engs[et % 2], et, 1024, 2048, False)
        nc.sync.dma_start(out=wts[:, :, 128:256], in_=wtr[:, :, 128:256])  # pack B
        # other half (gates B0-B3, k-tiles 16-31)
        for blk in range(2):
            for et in range(NE):
                in_piece(late_engs[et % 2], et, blk * 1024, blk * 1024 + 1024, True)

        # --- helpers ---
        def proj_half(w0, wm, dst, hh, oth=False, heat=False):
            acc = psS.tile([128, 1024], FP32, tag="s")
            x0 = (HALF if oth else 0) + hh * 512
            for et in range(NE):
                if heat:
                    # prologue heater: the input pieces trickle in while the
                    # HAM sustained-busy counter resets on every DMA wait;
                    # these keep the PE warm between piece arrivals. They
                    # write the acc tile's unused second bank.
                    ht = acc[:, 512:1024].bitcast(BF16)
                    nc.tensor.transpose(
                        out=ht[0:128, 0:128], in_=identB[:, :], identity=identB[:, :]
                    )
                nc.tensor.matmul(
                    out=acc[0:wm, 0:512],
                    lhsT=wts[:, et, w0 : w0 + wm],
                    rhs=xt[:, et, x0 : x0 + 512],
                    start=(et == 0),
                    stop=(et == NE - 1),
                )
            nc.vector.tensor_copy(
                out=dst[:, hh * 512 : (hh + 1) * 512], in_=acc[0:wm, 0:512]
            )

        def shift_q(hh):
            eng = nc.scalar if hh < 2 else nc.sync
            eng.dma_start(
                out=qts[:, hh * 512 : (hh + 1) * 512],
                in_=qk[64:128, hh * 512 : (hh + 1) * 512],
            )

        def shift_kos(jj):
            # K_oth odd tile j=2*jj+1 from base 0 (kv) to base 64 (kos)
            j = 2 * jj + 1
            eng = nc.sync
            eng.dma_start(
                out=kos[64:128, jj, :], in_=kv[0:64, j * 128 : (j + 1) * 128]
            )

        def vtr_burst(t0, n):
            # transpose V k-tiles t0..t0+n-1 into vn via one PSUM tile
            tp = psS.tile([128, 1024], BF16, tag="s")
            for i in range(n):
                t = t0 + i
                if t < 16:
                    src = ck[0:64, t * 128 : (t + 1) * 128]
                    idn = identB[0:64, 0:64]
                else:
                    j = t - 16
                    src = kv[64:128, j * 128 : (j + 1) * 128]
                    idn = identB[64:128, 64:128]
                nc.tensor.transpose(
                    out=tp[0:128, i * 64 : (i + 1) * 64], in_=src, identity=idn
                )
            for i in range(n):
                nc.vector.tensor_copy(
                    out=vn[:, t0 + i, 0:D], in_=tp[0:128, i * 64 : (i + 1) * 64]
                )

        def k_lhs0(t):
            if t < 16:
                return qk[0:64, t * 128 : (t + 1) * 128]
            j = t - 16
            return kv[0:64, j * 128 : (j + 1) * 128]

        def k_lhs64(t):
            if t < 16:
                return ck[64:128, t * 128 : (t + 1) * 128]
            j = t - 16
            return kos[64:128, (j - 1) // 2, :]

        def pick(g):
            # b needs a base-64 lhsT: any own tile (ck), or odd other tile
            # (kos). Prefer own[-2]: own[-1]'s C-projection may land only in
            # this group's own side slot (C halves trail A halves by one).
            cand = [t for t in g if t < 16 or ((t - 16) % 2 == 1)]
            own = [t for t in cand if t < 16]
            if len(own) >= 2:
                b = own[-2]
            elif own:
                b = own[-1]
            else:
                b = cand[-1]
            rest = [t for t in g if t != b]
            return rest[0], b, (rest[1] if len(rest) > 1 else None)

        # --- prologue: projections for own k-tiles 0-7 + pass-0 q ---
        proj_half(0, 128, qk, 0, heat=True)
        shift_q(0)
        proj_half(0, 128, qk, 1, heat=True)
        shift_q(1)
        proj_half(256, 128, ck, 0, heat=True)
        vtr_burst(0, 4)

        # side-work schedule for pass 0 (gi -> list of thunks)
        SIDE = {
            0: [lambda: proj_half(256, 128, ck, 1), lambda: vtr_burst(4, 4)],
            1: [lambda: proj_half(0, 128, qk, 2), lambda: shift_q(2)],
            2: [lambda: proj_half(256, 128, ck, 2), lambda: vtr_burst(8, 4)],
            3: [lambda: proj_half(0, 128, qk, 3), lambda: shift_q(3),
                lambda: proj_half(256, 128, ck, 3)],
            4: [lambda: proj_half(128, 128, kv, 0, True), lambda: vtr_burst(12, 4)],
            5: [lambda: proj_half(128, 128, kv, 1, True), lambda: shift_kos(0),
                lambda: shift_kos(1), lambda: vtr_burst(16, 4)],
            6: [lambda: proj_half(128, 128, kv, 2, True), lambda: shift_kos(2),
                lambda: shift_kos(3), lambda: vtr_burst(20, 4)],
            7: [lambda: proj_half(128, 128, kv, 3, True), lambda: shift_kos(4),
                lambda: shift_kos(5), lambda: vtr_burst(24, 4)],
            8: [lambda: shift_kos(6), lambda: shift_kos(7),
                lambda: vtr_burst(28, 4)],
        }

        GROUPS = [list(range(i, i + 3)) for i in range(0, 30, 3)] + [[30, 31]]

        out_engs = [nc.sync, nc.scalar]

        for ps in range(2):
            q0 = ps * 1024
            att = psA.tile([128, 1024], FP32)
            state = {"n": 0}
            prev_pair = None
            prev_sgl = None

            def av(t, pt, off, state=state, att=att):
                first = state["n"] == 0
                state["n"] += 1
                last = state["n"] == NT
                for c in range(2):
                    nc.tensor.matmul(
                        out=att[0:65, c * 512 : (c + 1) * 512],
                        lhsT=vn[:, t, :],
                        rhs=pt[:, off + c * 512 : off + (c + 1) * 512],
                        start=first,
                        stop=last,
                        skip_group_check=True,
                    )

            for gi, g in enumerate(GROUPS):
                a, b, s = pick(g)
                P = psP.tile([128, 2048], FP32, tag="p")
                if ps == 1 and gi >= 1:
                    # pass-1 heater: exp-paced groups leave ~25% PE idle in
                    # sub-us slivers; HAM oscillates without these (K18).
                    hp = P[:, 0:1024].bitcast(BF16)
                    for _ in range(2):
                        nc.tensor.transpose(
                            out=hp[0:128, 0:128], in_=identB[:, :], identity=identB[:, :]
                        )
                for c in range(2):
                    nc.tensor.matmul(
                        out=P[:, c * 512 : (c + 1) * 512],
                        lhsT=k_lhs0(a),
                        rhs=qts[:, q0 + c * 512 : q0 + (c + 1) * 512],
                        start=True,
                        stop=True,
                    )
                    nc.tensor.matmul(
                        out=P[:, 1024 + c * 512 : 1024 + (c + 1) * 512],
                        lhsT=k_lhs64(b),
                        rhs=qk[64:128, q0 + c * 512 : q0 + (c + 1) * 512],
                        start=True,
                        stop=True,
                    )
                pP = ppP.tile([128, 2048], BF16)
                nc.scalar.activation(out=pP[:, :], in_=P[:, :], func=Exp, scale=SCALE)

                if prev_pair is not None:
                    pa, pb, ppt = prev_pair
                    av(pa, ppt, 0)
                    av(pb, ppt, 1024)
                prev_pair = (a, b, pP)

                # side work BEFORE the single-scores: its psS allocation then
                # only waits on expS(g-1) (already done), not expS(g) — the
                # PE FIFO never idles behind the Scalar engine.
                if ps == 0:
                    for thunk in SIDE.get(gi, ()):
                        thunk()

                if s is not None:
                    Sc = psS.tile([128, 1024], FP32, tag="s")
                    for c in range(2):
                        nc.tensor.matmul(
                            out=Sc[:, c * 512 : (c + 1) * 512],
                            lhsT=k_lhs0(s),
                            rhs=qts[:, q0 + c * 512 : q0 + (c + 1) * 512],
                            start=True,
                            stop=True,
                        )
                    pS = ppS.tile([128, 1024], BF16)
                    nc.scalar.activation(out=pS[:, :], in_=Sc[:, :], func=Exp, scale=SCALE)
                else:
                    pS = None

                if prev_sgl is not None:
                    av(prev_sgl[0], prev_sgl[1], 0)
                prev_sgl = (s, pS) if s is not None else None

            pa, pb, ppt = prev_pair
            av(pa, ppt, 0)
            av(pb, ppt, 1024)
            if prev_sgl is not None:
                av(prev_sgl[0], prev_sgl[1], 0)

            for c in range(2):
                cols = slice(q0 + c * 512, q0 + (c + 1) * 512)
                nc.vector.tensor_copy(
                    out=att_sb[:, cols], in_=att[0:65, c * 512 : (c + 1) * 512]
                )
                out_engs[c].dma_start(out=out_d[:, cols], in_=att_sb[:, cols])

    nc.compile()
    _CACHE["nc"] = nc
    return nc


def _make_in_maps(x, Wq, Wk, Wv):
    import ml_dtypes

    bf16 = ml_dtypes.bfloat16
    xT = np.ascontiguousarray(x.transpose(0, 2, 1)).astype(bf16)  # [B, E, S]
    wt = np.concatenate(
        [Wk.T, Wq.T, Wk.T, Wv.T, Wv.T, Wk.T], axis=1
    ).astype(bf16)  # [E, 384]
    in_maps = []
    for c in range(N_CORES):
        b, h = divmod(c, 2)
        in_maps.append(
            {
                "xt_q": np.ascontiguousarray(xT[b, :, h * HALF : (h + 1) * HALF]),
                "xt_o": np.ascontiguousarray(
                    xT[b, :, (1 - h) * HALF : (2 - h) * HALF]
                ),
                "wt": wt,
            }
        )
    return in_maps


def _run(x, Wq, Wk, Wv, trace=False):
    from concourse.bass_utils import run_bass_kernel_spmd

    nc = _build()
    in_maps = _make_in_maps(x, Wq, Wk, Wv)
    res = run_bass_kernel_spmd(
        nc, in_maps, core_ids=list(range(N_CORES)), trace=trace
    )
    out = np.empty((B, S, D), dtype=np.float32)
    for c in range(N_CORES):
        b, h = divmod(c, 2)
        att = res.results[c]["out"]  # [65, HALF]: attn^T rows + denom row
        out[b, h * HALF : (h + 1) * HALF, :] = (att[0:D] / att[D : D + 1]).T
    return out, res


def kernel(x, Wq, Wk, Wv):
    out, _ = _run(
        np.asarray(x, dtype=np.float32),
        np.asarray(Wq, dtype=np.float32),
        np.asarray(Wk, dtype=np.float32),
        np.asarray(Wv, dtype=np.float32),
    )
    return out


# revision 15
# speedup vs baseline: 1.0288x; 1.0288x over previous
"""Single-head attention (B=4, S=4096, E=1024, D=64) on 8 TRN2 NeuronCores.

Sharding: data-parallel over (batch, query-half): core c handles batch
b = c // 2 and query rows [h*2048, (h+1)*2048) with h = c % 2. Each core
computes Q for its own 2048 rows and K/V for the full 4096 rows of its batch.

Design (vs the v1 kernel this evolved from):
  * Row-tiled scores: the scores matmul contracts over D=64 (half the PE
    array), so two k-tiles run CONCURRENTLY as PE row-tiles (0,0)/(64,0),
    with lhsT/rhs at base partitions 0 and 64 (tile_position auto-derives).
    K is produced at BOTH bases for free by packing the projections as
    A=[Wk|Wq]own, C=[Wv|Wk]own (pass C's top half was idle in v1) and
    B=[Wk|Wv]oth, plus 8 small base-shifts for odd other-half tiles (kos).
    The (64,0) tiles consume Q at base 64 straight from the pass-A layout;
    the (0,0) tiles consume the base-0 shifted copy (qts).
  * Paired exp: each k-tile pair's scores land in one [128, 2048] PSUM
    tile (4 banks) -> ONE ScalarE activation per 2 k-tiles, amortizing the
    ~485ns per-activation overhead. PSUM: pair(4) + single(2) + att(2)
    banks; groups of 3 k-tiles = one pair + one single, double-buffered
    through the two scores slots. ScalarE: ~85.6us -> ~67us.
  * Per group the PE issues: pair scores, [side work], single scores, and
    the PREVIOUS group's AV matmuls. Side work (projection halves and
    PE-transpose bursts for V) is issued BEFORE the single so its PSUM
    allocation waits only on the previous group's single-exp; issuing it
    after would head-of-line-block the PE FIFO on the current exp.
  * Own-half k-tiles first within each pass so compute starts once the own
    half of x lands (~13us); the other half streams in under the loop.
  * HAM p-state care: warm-up transposes cover the DMA spin-up, heater
    transposes bridge piece-arrival gaps inside the prologue projections
    and the exp-paced pass-1 groups (sub-us PE idle slivers otherwise
    oscillate the clock down to 4/8).
  * attn rides with a ones-row in the AV stationary ([V|1], M=65): row 64
    accumulates softmax denominators; the host divides and transposes.
"""

import numpy as np

B, S, E, D = 4, 4096, 1024, 64
HALF = S // 2
N_CORES = 8
SCALE = 1.0 / np.sqrt(D)

NE = E // 128  # 8 e-tiles (contraction)
NT = S // 128  # 32 k-tiles per batch
N_WARM = 40

_CACHE = {}


def _build():
    if "nc" in _CACHE:
        return _CACHE["nc"]

    from contextlib import ExitStack

    import concourse.bacc as bacc
    import concourse.tile as tile
    from concourse import mybir
    from concourse.masks import make_identity

    FP32 = mybir.dt.float32
    BF16 = mybir.dt.bfloat16
    Exp = mybir.ActivationFunctionType.Exp

    nc = bacc.Bacc(
        "TRN2", target_bir_lowering=False, debug=False, num_devices=N_CORES
    )

    xt_q_d = nc.dram_tensor("xt_q", [E, HALF], BF16, kind="ExternalInput").ap()
    xt_o_d = nc.dram_tensor("xt_o", [E, HALF], BF16, kind="ExternalInput").ap()
    wt_d = nc.dram_tensor("wt", [E, 384], BF16, kind="ExternalInput").ap()
    out_d = nc.dram_tensor("out", [D + 1, HALF], FP32, kind="ExternalOutput").ap()

    with tile.TileContext(nc) as tc, ExitStack() as ctx:
        const = ctx.enter_context(tc.tile_pool(name="const", bufs=1))
        big = ctx.enter_context(tc.tile_pool(name="big", bufs=1))
        ppP = ctx.enter_context(tc.tile_pool(name="ppP", bufs=2))
        ppS = ctx.enter_context(tc.tile_pool(name="ppS", bufs=2))
        psP = ctx.enter_context(tc.tile_pool(name="psP", bufs=1, space="PSUM"))
        psS = ctx.enter_context(tc.tile_pool(name="psS", bufs=1, space="PSUM"))
        psA = ctx.enter_context(tc.tile_pool(name="psA", bufs=1, space="PSUM"))

        identB = const.tile([128, 128], BF16)
        make_identity(nc, identB)

        xt = big.tile([128, NE, S], BF16)  # x^T; cols [0, HALF) own
        wts = big.tile([128, NE, 384], BF16)  # [Wk|Wq | Wk|Wv | Wv|Wk]
        qk = big.tile([128, HALF], BF16)  # K_own@0-63 | Q_own@64-127
        ck = big.tile([128, HALF], BF16)  # V_own@0-63 | K_own@64-127
        kv = big.tile([128, HALF], BF16)  # K_oth@0-63 | V_oth@64-127
        qts = big.tile([64, HALF], BF16)  # Q_own shifted to base 0
        kos = big.tile([128, 8, 128], BF16)  # rows 64:128 = K_oth odd tiles
        vn = big.tile([128, NT, D + 1], BF16)  # V natural + ones col
        att_sb = big.tile([65, HALF], FP32)

        # --- PE warm-up over the DMA spin-up window ---
        warm = psS.tile([128, 1024], BF16, tag="s")
        for _ in range(N_WARM):
            nc.tensor.transpose(
                out=warm[0:128, 0:128], in_=identB[:, :], identity=identB[:, :]
            )

        # --- input DMAs: priority order per engine queue ---
        wtr = wt_d.rearrange("(t p) d -> p t d", p=128)

        def in_piece(eng, et, c0, c1, oth):
            src, off = (xt_o_d, HALF) if oth else (xt_q_d, 0)
            eng.dma_start(
                out=xt[:, et, off + c0 : off + c1],
                in_=src[et * 128 : (et + 1) * 128, c0:c1],
            )

        # ones columns of vn (vector queue is free until the first proj copy)
        for t in range(NT):
            nc.vector.memset(vn[:, t, D : D + 1], 1.0)

        nc.sync.dma_start(out=wts[:, :, 0:128], in_=wtr[:, :, 0:128])  # pack A
        # scalar gets NO input pieces: its in-order queue must stay clean for
        # the latency-critical qts shifts (g0's scores block on them; pieces
        # ahead of the shifts cost ~4.5us of PE idle).
        first_engs = [nc.gpsimd, nc.sync]
        # own cols 0:1024 in 512-col pieces (gates prologue + pass-0 q)
        for blk in range(2):
            for et in range(NE):
                in_piece(first_engs[et % 2], et, blk * 512, blk * 512 + 512, False)
        nc.gpsimd.dma_start(out=wts[:, :, 256:384], in_=wtr[:, :, 256:384])  # pack C
        late_engs = [nc.gpsimd, nc.sync]
        # own cols 1024:2048 (gates A2/A3/C2/C3 + pass-1 q)
        for et in range(NE):
            in_piece(late_engs[et % 2], et, 1024, 2048, False)
        nc.sync.dma_start(out=wts[:, :, 128:256], in_=wtr[:, :, 128:256])  # pack B
        # other half (gates B0-B3, k-tiles 16-31)
        for blk in range(2):
            for et in range(NE):
                in_piece(late_engs[et % 2], et, blk * 1024, blk * 1024 + 1024, True)

        # --- helpers ---
        def proj_half(w0, wm, dst, hh, oth=False, heat=False):
            acc = psS.tile([128, 1024], FP32, tag="s")
            x0 = (HALF if oth else 0) + hh * 512
            for et in range(NE):
                if heat:
                    # prologue heater: the input pieces trickle in while the
                    # HAM sustained-busy counter resets on every DMA wait;
                    # these keep the PE warm between piece arrivals. They
                    # write the acc tile's unused second bank.
                    ht = acc[:, 512:1024].bitcast(BF16)
                    nc.tensor.transpose(
                        out=ht[0:128, 0:128], in_=identB[:, :], identity=identB[:, :]
                    )
                nc.tensor.matmul(
                    out=acc[0:wm, 0:512],
                    lhsT=wts[:, et, w0 : w0 + wm],
                    rhs=xt[:, et, x0 : x0 + 512],
                    start=(et == 0),
                    stop=(et == NE - 1),
                )
            nc.vector.tensor_copy(
                out=dst[:, hh * 512 : (hh + 1) * 512], in_=acc[0:wm, 0:512]
            )

        def shift_q(hh):
            eng = nc.scalar if hh < 2 else nc.sync
            eng.dma_start(
                out=qts[:, hh * 512 : (hh + 1) * 512],
                in_=qk[64:128, hh * 512 : (hh + 1) * 512],
            )

        def shift_kos(jj):
            # K_oth odd tile j=2*jj+1 from base 0 (kv) to base 64 (kos)
            j = 2 * jj + 1
            eng = nc.sync
            eng.dma_start(
                out=kos[64:128, jj, :], in_=kv[0:64, j * 128 : (j + 1) * 128]
            )

        def vtr_burst(t0, n):
            # transpose V k-tiles t0..t0+n-1 into vn via one PSUM tile
            tp = psS.tile([128, 1024], BF16, tag="s")
            for i in range(n):
                t = t0 + i
                if t < 16:
                    src = ck[0:64, t * 128 : (t + 1) * 128]
                    idn = identB[0:64, 0:64]
                else:
                    j = t - 16
                    src = kv[64:128, j * 128 : (j + 1) * 128]
                    idn = identB[64:128, 64:128]
                nc.tensor.transpose(
                    out=tp[0:128, i * 64 : (i + 1) * 64], in_=src, identity=idn
                )
            for i in range(n):
                nc.vector.tensor_copy(
                    out=vn[:, t0 + i, 0:D], in_=tp[0:128, i * 64 : (i + 1) * 64]
                )

        def k_lhs0(t):
            if t < 16:
                return qk[0:64, t * 128 : (t + 1) * 128]
            j = t - 16
            return kv[0:64, j * 128 : (j + 1) * 128]

        def k_lhs64(t):
            if t < 16:
                return ck[64:128, t * 128 : (t + 1) * 128]
            j = t - 16
            return kos[64:128, (j - 1) // 2, :]

        def pick(g):
            # b needs a base-64 lhsT: any own tile (ck), or odd other tile
            # (kos). Prefer own[-2]: own[-1]'s C-projection may land only in
            # this group's own side slot (C halves trail A halves by one).
            cand = [t for t in g if t < 16 or ((t - 16) % 2 == 1)]
            own = [t for t in cand if t < 16]
            if len(own) >= 2:
                b = own[-2]
            elif own:
                b = own[-1]
            else:
                b = cand[-1]
            rest = [t for t in g if t != b]
            return rest[0], b, (rest[1] if len(rest) > 1 else None)

        # --- prologue: projections for own k-tiles 0-7 + pass-0 q ---
        proj_half(0, 128, qk, 0, heat=True)
        shift_q(0)
        proj_half(0, 128, qk, 1, heat=True)
        shift_q(1)
        proj_half(256, 128, ck, 0, heat=True)
        vtr_burst(0, 4)

        # side-work schedule for pass 0 (gi -> list of thunks)
        SIDE = {
            0: [lambda: proj_half(256, 128, ck, 1), lambda: vtr_burst(4, 4)],
            1: [lambda: proj_half(0, 128, qk, 2), lambda: shift_q(2)],
            2: [lambda: proj_half(256, 128, ck, 2), lambda: vtr_burst(8, 4)],
            3: [lambda: proj_half(0, 128, qk, 3), lambda: shift_q(3),
                lambda: proj_half(256, 128, ck, 3)],
            4: [lambda: proj_half(128, 128, kv, 0, True), lambda: vtr_burst(12, 4)],
            5: [lambda: proj_half(128, 128, kv, 1, True), lambda: shift_kos(0),
                lambda: shift_kos(1), lambda: vtr_burst(16, 4)],
            6: [lambda: proj_half(128, 128, kv, 2, True), lambda: shift_kos(2),
                lambda: shift_kos(3), lambda: vtr_burst(20, 4)],
            7: [lambda: proj_half(128, 128, kv, 3, True), lambda: shift_kos(4),
                lambda: shift_kos(5), lambda: vtr_burst(24, 4)],
            8: [lambda: shift_kos(6), lambda: shift_kos(7),
                lambda: vtr_burst(28, 4)],
        }

        GROUPS = [list(range(i, i + 3)) for i in range(0, 30, 3)] + [[30, 31]]

        out_engs = [nc.sync, nc.scalar]

        for ps in range(2):
            q0 = ps * 1024
            att = psA.tile([128, 1024], FP32)
            state = {"n": 0}
            prev_pair = None
            prev_sgl = None

            def av(t, pt, off, state=state, att=att):
                first = state["n"] == 0
                state["n"] += 1
                last = state["n"] == NT
                for c in range(2):
                    nc.tensor.matmul(
                        out=att[0:65, c * 512 : (c + 1) * 512],
                        lhsT=vn[:, t, :],
                        rhs=pt[:, off + c * 512 : off + (c + 1) * 512],
                        start=first,
                        stop=last,
                        skip_group_check=True,
                    )

            for gi, g in enumerate(GROUPS):
                a, b, s = pick(g)
                P = psP.tile([128, 2048], FP32, tag="p")
                if ps == 1 and gi >= 1:
                    # pass-1 heater: exp-paced groups leave ~25% PE idle in
                    # sub-us slivers; HAM oscillates without these (K18).
                    hp = P[:, 0:1024].bitcast(BF16)
                    for _ in range(2):
                        nc.tensor.transpose(
                            out=hp[0:128, 0:128], in_=identB[:, :], identity=identB[:, :]
                        )
                for c in range(2):
                    nc.tensor.matmul(
                        out=P[:, c * 512 : (c + 1) * 512],
                        lhsT=k_lhs0(a),
                        rhs=qts[:, q0 + c * 512 : q0 + (c + 1) * 512],
                        start=True,
                        stop=True,
                    )
                    nc.tensor.matmul(
                        out=P[:, 1024 + c * 512 : 1024 + (c + 1) * 512],
                        lhsT=k_lhs64(b),
                        rhs=qk[64:128, q0 + c * 512 : q0 + (c + 1) * 512],
                        start=True,
                        stop=True,
                    )
                pP = ppP.tile([128, 2048], BF16)
                nc.scalar.activation(out=pP[:, :], in_=P[:, :], func=Exp, scale=SCALE)

                if prev_pair is not None:
                    pa, pb, ppt = prev_pair
                    av(pa, ppt, 0)
                    av(pb, ppt, 1024)
                prev_pair = (a, b, pP)

                # side work BEFORE the single-scores: its psS allocation then
                # only waits on expS(g-1) (already done), not expS(g) — the
                # PE FIFO never idles behind the Scalar engine.
                if ps == 0:
                    for thunk in SIDE.get(gi, ()):
                        thunk()

                if s is not None:
                    Sc = psS.tile([128, 1024], FP32, tag="s")
                    for c in range(2):
                        nc.tensor.matmul(
                            out=Sc[:, c * 512 : (c + 1) * 512],
                            lhsT=k_lhs0(s),
                            rhs=qts[:, q0 + c * 512 : q0 + (c + 1) * 512],
                            start=True,
                            stop=True,
                        )
                    pS = ppS.tile([128, 1024], BF16)
                    nc.scalar.activation(out=pS[:, :], in_=Sc[:, :], func=Exp, scale=SCALE)
                else:
                    pS = None

                if prev_sgl is not None:
                    av(prev_sgl[0], prev_sgl[1], 0)
                prev_sgl = (s, pS) if s is not None else None

            pa, pb, ppt = prev_pair
            av(pa, ppt, 0)
            av(pb, ppt, 1024)
            if prev_sgl is not None:
                av(prev_sgl[0], prev_sgl[1], 0)

            for c in range(2):
                cols = slice(q0 + c * 512, q0 + (c + 1) * 512)
                nc.vector.tensor_copy(
                    out=att_sb[:, cols], in_=att[0:65, c * 512 : (c + 1) * 512]
                )
                out_engs[c].dma_start(out=out_d[:, cols], in_=att_sb[:, cols])

    nc.compile()
    _CACHE["nc"] = nc
    return nc


def _make_in_maps(x, Wq, Wk, Wv):
    import ml_dtypes

    bf16 = ml_dtypes.bfloat16
    xT = np.ascontiguousarray(x.transpose(0, 2, 1)).astype(bf16)  # [B, E, S]
    wt = np.concatenate(
        [Wk.T, Wq.T, Wk.T, Wv.T, Wv.T, Wk.T], axis=1
    ).astype(bf16)  # [E, 384]
    in_maps = []
    for c in range(N_CORES):
        b, h = divmod(c, 2)
        in_maps.append(
            {
                "xt_q": np.ascontiguousarray(xT[b, :, h * HALF : (h + 1) * HALF]),
                "xt_o": np.ascontiguousarray(
                    xT[b, :, (1 - h) * HALF : (2 - h) * HALF]
                ),
                "wt": wt,
            }
        )
    return in_maps


def _run(x, Wq, Wk, Wv, trace=False):
    from concourse.bass_utils import run_bass_kernel_spmd

    nc = _build()
    in_maps = _make_in_maps(x, Wq, Wk, Wv)
    res = run_bass_kernel_spmd(
        nc, in_maps, core_ids=list(range(N_CORES)), trace=trace
    )
    out = np.empty((B, S, D), dtype=np.float32)
    for c in range(N_CORES):
        b, h = divmod(c, 2)
        att = res.results[c]["out"]  # [65, HALF]: attn^T rows + denom row
        out[b, h * HALF : (h + 1) * HALF, :] = (att[0:D] / att[D : D + 1]).T
    return out, res


def kernel(x, Wq, Wk, Wv):
    out, _ = _run(
        np.asarray(x, dtype=np.float32),
        np.asarray(Wq, dtype=np.float32),
        np.asarray(Wk, dtype=np.float32),
        np.asarray(Wv, dtype=np.float32),
    )
    return out
